# revision 12
# baseline (speedup 1.0000x reference)
"""Trainium2 Bass kernel for nn_AttentionMapLayer.

Computes out[b,h,w,c] = (l2n(s_o)[b,w] * l2n(t_o)[b,h] + roi[h,w]) * ipt[b,h,w,c]
where l2n is tf-style l2_normalize (x * rsqrt(max(sum(x^2), 1e-12))).

Sharding: pure data parallel over batch (16) across 8 NeuronCores, 2 batches
per core; roi_map replicated. Per core the kernel is HBM-bandwidth bound:
~30.7 MB read (ipt shard) + ~30.7 MB written (out shard).

Per-core structure (v2):
  - (b, h) flattened to 600 rows; ipt/out declared as [600, 25, 512] so
    stream tiles use the full 128 partitions (=> all 16 SDMA engines; the
    v1 100-partition tiles only engaged 10 engines, ~174 GB/s).
  - l2-normalization factors on 1-partition tiles; both rsqrt factors folded
    into s: s_hat = s_o * rs_s * rs_t, so a = s_hat (outer) t_o_raw + roi.
  - s_hat broadcast across 128 partitions via gpsimd.partition_broadcast;
    t_o loaded column-wise (rows on partitions) straight from DRAM; per-row-
    tile/per-batch-segment a = s_bc * t_col + roi on DVE.
    (A K=1 TensorE matmul outer product wedges the device; tensor_tensor_
    reduce also wedges it — both avoided.)
  - main stream: row tiles [128,128,128,128,88] x w-chunks (13|12)*512 free;
    DMA in on SyncE HWDGE queue, per-w tensor_scalar multiply by a[:, w]
    (per-partition scalar) in place, DMA out on ScalarE HWDGE queue (second
    hardware queue keeps all SDMA engines fed).
"""

import os
import sys

import numpy as np

for _p in (
    "/root/.axon_site",
    "/root/.axon_site/_ro/trn_rl_repo",
    "/root/.axon_site/_ro/pypackages",
    "/opt/trn_rl_repo",
):
    if os.path.isdir(_p) and _p not in sys.path:
        sys.path.append(_p)

import concourse.bacc as bacc
import concourse.bass as bass
import concourse.tile as tile
from concourse import mybir
from concourse.bass_utils import run_bass_kernel_spmd

N_CORES = 8
B, H, W, C = 16, 300, 25, 512
NB = B // N_CORES   # batches per core
NR = NB * H         # flattened rows per core
ROW_TILES = ((0, 128), (128, 128), (256, 128), (384, 128), (512, 88))
W_SPLITS = ((0, 25),)
EPS = 1e-12

_NC_CACHE = []


def _segments(r0, plen):
    """Split rows [r0, r0+plen) at batch boundaries -> (p0, b, h0, seglen)."""
    segs = []
    r = r0
    while r < r0 + plen:
        b, h0 = divmod(r, H)
        seglen = min(r0 + plen - r, H - h0)
        segs.append((r - r0, b, h0, seglen))
        r += seglen
    return segs


def _build():
    dt = mybir.dt.float32
    nc = bacc.Bacc(None)
    s_o = nc.declare_dram_parameter("s_o", [NB, W], dt, isOutput=False)
    t_o = nc.declare_dram_parameter("t_o", [NB, H], dt, isOutput=False)
    ipt = nc.declare_dram_parameter("ipt", [NR, W, C], dt, isOutput=False)
    roi = nc.declare_dram_parameter("roi_map", [1, H, W], dt, isOutput=False)
    out = nc.declare_dram_parameter("out", [NR, W, C], dt, isOutput=True)

    mult = mybir.AluOpType.mult
    NT = len(ROW_TILES)

    with tile.TileContext(nc) as tc:
        with (
            tc.tile_pool(name="small", bufs=1) as small,
            tc.tile_pool(name="dram", bufs=1, space="DRAM") as dram,
            tc.tile_pool(name="big", bufs=3) as big,
        ):
            roi_sb = small.tile([128, NT, W], dt)
            a_sb = small.tile([128, NT, W], dt)
            s_row = small.tile([128, NT, W], dt)
            t_col = small.tile([128, NT], dt)
            s_hat_d = dram.tile([NB, W], dt)
            # per-batch 1-partition tiles for the normalization factors
            s_sb = [small.tile([1, W], dt, name=f"s{b}", tag=f"s{b}") for b in range(NB)]
            t_sb = [small.tile([1, H], dt, name=f"t{b}", tag=f"t{b}") for b in range(NB)]
            sq_s = [small.tile([1, W], dt, name=f"qs{b}", tag=f"qs{b}") for b in range(NB)]
            sq_t = [small.tile([1, H], dt, name=f"qt{b}", tag=f"qt{b}") for b in range(NB)]
            rs_s = [small.tile([1, 1], dt, name=f"rs{b}", tag=f"rs{b}") for b in range(NB)]
            rs_t = [small.tile([1, 1], dt, name=f"rt{b}", tag=f"rt{b}") for b in range(NB)]

            for b in range(NB):
                nc.sync.dma_start(out=s_sb[b][:], in_=s_o[b : b + 1, :])
                nc.sync.dma_start(out=t_sb[b][:], in_=t_o[b : b + 1, :])
            for rt, (r0, plen) in enumerate(ROW_TILES):
                for p0, b, h0, seglen in _segments(r0, plen):
                    nc.sync.dma_start(
                        out=t_col[p0 : p0 + seglen, rt : rt + 1],
                        in_=t_o[b, h0 : h0 + seglen],
                    )
                    nc.sync.dma_start(
                        out=roi_sb[p0 : p0 + seglen, rt, :],
                        in_=roi[0, h0 : h0 + seglen, :],
                    )

            # rs = 1/sqrt(max(sum(x^2), eps)) per vector; fold both into s:
            # s_hat = s_o * rs_s * rs_t  (so a = s_hat (outer) t_o + roi)
            for b in range(NB):
                for sq, sb, rs in (
                    (sq_s[b], s_sb[b], rs_s[b]),
                    (sq_t[b], t_sb[b], rs_t[b]),
                ):
                    nc.vector.tensor_mul(out=sq[:], in0=sb[:], in1=sb[:])
                    nc.vector.reduce_sum(
                        out=rs[:], in_=sq[:], axis=mybir.AxisListType.X
                    )
                    nc.vector.tensor_scalar_max(out=rs[:], in0=rs[:], scalar1=EPS)
                    nc.scalar.sqrt(out=rs[:], in_=rs[:])
                    nc.vector.reciprocal(out=rs[:], in_=rs[:])
                nc.vector.tensor_scalar(
                    out=s_sb[b][:], in0=s_sb[b][:], scalar1=rs_s[b][:],
                    scalar2=rs_t[b][:], op0=mult, op1=mult,
                )
                nc.sync.dma_start(out=s_hat_d[b : b + 1, :], in_=s_sb[b][:])

            # s_row[p, rt, :] = s_hat[b(row)] via partition-stride-0 DMA bcast
            for rt, (r0, plen) in enumerate(ROW_TILES):
                for p0, b, h0, seglen in _segments(r0, plen):
                    base = s_hat_d[b, :]
                    bcast = bass.AP(
                        tensor=base.tensor,
                        offset=base.offset,
                        ap=[[0, seglen]] + list(base.ap),
                    )
                    nc.sync.dma_start(
                        out=s_row[p0 : p0 + seglen, rt, :], in_=bcast
                    )

            # a[:, rt, :] = s_row * t_col + roi (full-width DVE, start part 0)
            for rt, (r0, plen) in enumerate(ROW_TILES):
                nc.vector.tensor_scalar_mul(
                    out=a_sb[:plen, rt, :], in0=s_row[:plen, rt, :],
                    scalar1=t_col[:plen, rt : rt + 1],
                )
                nc.vector.tensor_add(
                    out=a_sb[:plen, rt, :], in0=a_sb[:plen, rt, :],
                    in1=roi_sb[:plen, rt, :],
                )

            # main bandwidth-bound stream: in on SyncE queue, out on ScalarE queue
            for rt, (r0, plen) in enumerate(ROW_TILES):
                for w0, w1 in W_SPLITS:
                    nw = w1 - w0
                    t = big.tile([128, 25, C], dt, name="stream", tag="stream")
                    nc.sync.dma_start(
                        out=t[:plen, :nw, :], in_=ipt[r0 : r0 + plen, w0:w1, :]
                    )
                    for wi in range(nw):
                        nc.vector.tensor_scalar_mul(
                            out=t[:plen, wi, :],
                            in0=t[:plen, wi, :],
                            scalar1=a_sb[:plen, rt, w0 + wi : w0 + wi + 1],
                        )
                    nc.scalar.dma_start(
                        out=out[r0 : r0 + plen, w0:w1, :], in_=t[:plen, :nw, :]
                    )
    nc.finalize()
    return nc


def _get_nc():
    if not _NC_CACHE:
        _NC_CACHE.append(_build())
    return _NC_CACHE[0]


def _make_in_maps(s_o, t_o, ipt, roi_map):
    s_o = np.ascontiguousarray(np.asarray(s_o, dtype=np.float32))
    t_o = np.ascontiguousarray(np.asarray(t_o, dtype=np.float32))
    ipt = np.asarray(ipt, dtype=np.float32)
    roi_map = np.ascontiguousarray(np.asarray(roi_map, dtype=np.float32))
    in_maps = []
    for i in range(N_CORES):
        lo, hi = i * NB, (i + 1) * NB
        in_maps.append(
            {
                "s_o": s_o[lo:hi],
                "t_o": t_o[lo:hi],
                "ipt": np.ascontiguousarray(ipt[lo:hi]).reshape(NR, W, C),
                "roi_map": roi_map,
            }
        )
    return in_maps


def _execute(in_maps, **kwargs):
    nc = _get_nc()
    return run_bass_kernel_spmd(nc, in_maps, core_ids=list(range(N_CORES)), **kwargs)


def kernel(s_o, t_o, ipt, roi_map):
    in_maps = _make_in_maps(s_o, t_o, ipt, roi_map)
    res = _execute(in_maps)
    return np.concatenate(
        [res.results[i]["out"].reshape(NB, H, W, C) for i in range(N_CORES)], axis=0
    )


# revision 13
# speedup vs baseline: 1.0810x; 1.0810x over previous
"""Trainium2 Bass kernel for nn_AttentionMapLayer.

Computes out[b,h,w,c] = (l2n(s_o)[b,w] * l2n(t_o)[b,h] + roi[h,w]) * ipt[b,h,w,c]
where l2n is tf-style l2_normalize (x * rsqrt(max(sum(x^2), 1e-12))).

Sharding: pure data parallel over batch (16) across 8 NeuronCores, 2 batches
per core; roi_map replicated. Per core the kernel is HBM-bandwidth bound:
~30.7 MB read (ipt shard) + ~30.7 MB written (out shard).

Per-core structure (v2):
  - (b, h) flattened to 600 rows; ipt/out declared as [600, 25, 512] so
    stream tiles use the full 128 partitions (=> all 16 SDMA engines; the
    v1 100-partition tiles only engaged 10 engines, ~174 GB/s).
  - l2-normalization factors on 1-partition tiles; both rsqrt factors folded
    into s: s_hat = s_o * rs_s * rs_t, so a = s_hat (outer) t_o_raw + roi.
  - s_hat broadcast across 128 partitions via gpsimd.partition_broadcast;
    t_o loaded column-wise (rows on partitions) straight from DRAM; per-row-
    tile/per-batch-segment a = s_bc * t_col + roi on DVE.
    (A K=1 TensorE matmul outer product wedges the device; tensor_tensor_
    reduce also wedges it — both avoided.)
  - main stream: row tiles [128,128,128,128,88] x w-chunks (13|12)*512 free;
    DMA in on SyncE HWDGE queue, per-w tensor_scalar multiply by a[:, w]
    (per-partition scalar) in place, DMA out on ScalarE HWDGE queue (second
    hardware queue keeps all SDMA engines fed).
"""

import os
import sys

import numpy as np

for _p in (
    "/root/.axon_site",
    "/root/.axon_site/_ro/trn_rl_repo",
    "/root/.axon_site/_ro/pypackages",
    "/opt/trn_rl_repo",
):
    if os.path.isdir(_p) and _p not in sys.path:
        sys.path.append(_p)

import concourse.bacc as bacc
import concourse.bass as bass
import concourse.tile as tile
from concourse import mybir
from concourse.bass_utils import run_bass_kernel_spmd

N_CORES = 8
B, H, W, C = 16, 300, 25, 512
NB = B // N_CORES   # batches per core
NR = NB * H         # flattened rows per core
ROW_TILES = ((0, 128), (128, 128), (256, 128), (384, 128), (512, 88))
W_SPLITS = ((0, 9), (9, 17), (17, 25))
EPS = 1e-12

_NC_CACHE = []


def _segments(r0, plen):
    """Split rows [r0, r0+plen) at batch boundaries -> (p0, b, h0, seglen)."""
    segs = []
    r = r0
    while r < r0 + plen:
        b, h0 = divmod(r, H)
        seglen = min(r0 + plen - r, H - h0)
        segs.append((r - r0, b, h0, seglen))
        r += seglen
    return segs


def _build():
    dt = mybir.dt.float32
    nc = bacc.Bacc(None)
    s_o = nc.declare_dram_parameter("s_o", [NB, W], dt, isOutput=False)
    t_o = nc.declare_dram_parameter("t_o", [NB, H], dt, isOutput=False)
    ipt = nc.declare_dram_parameter("ipt", [NR, W, C], dt, isOutput=False)
    roi = nc.declare_dram_parameter("roi_map", [1, H, W], dt, isOutput=False)
    out = nc.declare_dram_parameter("out", [NR, W, C], dt, isOutput=True)

    mult = mybir.AluOpType.mult
    NT = len(ROW_TILES)

    with tile.TileContext(nc) as tc:
        with (
            tc.tile_pool(name="small", bufs=1) as small,
            tc.tile_pool(name="dram", bufs=1, space="DRAM") as dram,
            tc.tile_pool(name="big", bufs=6) as big,
        ):
            roi_sb = small.tile([128, NT, W], dt)
            a_sb = small.tile([128, NT, W], dt)
            s_row = small.tile([128, NT, W], dt)
            t_col = small.tile([128, NT], dt)
            s_hat_d = dram.tile([NB, W], dt)
            # per-batch 1-partition tiles for the normalization factors
            s_sb = [small.tile([1, W], dt, name=f"s{b}", tag=f"s{b}") for b in range(NB)]
            t_sb = [small.tile([1, H], dt, name=f"t{b}", tag=f"t{b}") for b in range(NB)]
            sq_s = [small.tile([1, W], dt, name=f"qs{b}", tag=f"qs{b}") for b in range(NB)]
            sq_t = [small.tile([1, H], dt, name=f"qt{b}", tag=f"qt{b}") for b in range(NB)]
            rs_s = [small.tile([1, 1], dt, name=f"rs{b}", tag=f"rs{b}") for b in range(NB)]
            rs_t = [small.tile([1, 1], dt, name=f"rt{b}", tag=f"rt{b}") for b in range(NB)]

            for b in range(NB):
                nc.sync.dma_start(out=s_sb[b][:], in_=s_o[b : b + 1, :])
                nc.sync.dma_start(out=t_sb[b][:], in_=t_o[b : b + 1, :])
            for rt, (r0, plen) in enumerate(ROW_TILES):
                for p0, b, h0, seglen in _segments(r0, plen):
                    nc.sync.dma_start(
                        out=t_col[p0 : p0 + seglen, rt : rt + 1],
                        in_=t_o[b, h0 : h0 + seglen],
                    )
                    nc.sync.dma_start(
                        out=roi_sb[p0 : p0 + seglen, rt, :],
                        in_=roi[0, h0 : h0 + seglen, :],
                    )

            # rs = 1/sqrt(max(sum(x^2), eps)) per vector; fold both into s:
            # s_hat = s_o * rs_s * rs_t  (so a = s_hat (outer) t_o + roi)
            for b in range(NB):
                for sq, sb, rs in (
                    (sq_s[b], s_sb[b], rs_s[b]),
                    (sq_t[b], t_sb[b], rs_t[b]),
                ):
                    nc.vector.tensor_mul(out=sq[:], in0=sb[:], in1=sb[:])
                    nc.vector.reduce_sum(
                        out=rs[:], in_=sq[:], axis=mybir.AxisListType.X
                    )
                    nc.vector.tensor_scalar_max(out=rs[:], in0=rs[:], scalar1=EPS)
                    nc.scalar.sqrt(out=rs[:], in_=rs[:])
                    nc.vector.reciprocal(out=rs[:], in_=rs[:])
                nc.vector.tensor_scalar(
                    out=s_sb[b][:], in0=s_sb[b][:], scalar1=rs_s[b][:],
                    scalar2=rs_t[b][:], op0=mult, op1=mult,
                )
                nc.sync.dma_start(out=s_hat_d[b : b + 1, :], in_=s_sb[b][:])

            # s_row[p, rt, :] = s_hat[b(row)] via partition-stride-0 DMA bcast
            for rt, (r0, plen) in enumerate(ROW_TILES):
                for p0, b, h0, seglen in _segments(r0, plen):
                    base = s_hat_d[b, :]
                    bcast = bass.AP(
                        tensor=base.tensor,
                        offset=base.offset,
                        ap=[[0, seglen]] + list(base.ap),
                    )
                    nc.sync.dma_start(
                        out=s_row[p0 : p0 + seglen, rt, :], in_=bcast
                    )

            # a[:, rt, :] = s_row * t_col + roi (full-width DVE, start part 0)
            for rt, (r0, plen) in enumerate(ROW_TILES):
                nc.vector.tensor_scalar_mul(
                    out=a_sb[:plen, rt, :], in0=s_row[:plen, rt, :],
                    scalar1=t_col[:plen, rt : rt + 1],
                )
                nc.vector.tensor_add(
                    out=a_sb[:plen, rt, :], in0=a_sb[:plen, rt, :],
                    in1=roi_sb[:plen, rt, :],
                )

            # main bandwidth-bound stream: in on SyncE queue, out on ScalarE queue
            for rt, (r0, plen) in enumerate(ROW_TILES):
                for w0, w1 in W_SPLITS:
                    nw = w1 - w0
                    t = big.tile([128, 9, C], dt, name="stream", tag="stream")
                    nc.sync.dma_start(
                        out=t[:plen, :nw, :], in_=ipt[r0 : r0 + plen, w0:w1, :]
                    )
                    for wi in range(nw):
                        nc.vector.tensor_scalar_mul(
                            out=t[:plen, wi, :],
                            in0=t[:plen, wi, :],
                            scalar1=a_sb[:plen, rt, w0 + wi : w0 + wi + 1],
                        )
                    nc.scalar.dma_start(
                        out=out[r0 : r0 + plen, w0:w1, :], in_=t[:plen, :nw, :]
                    )
    nc.finalize()
    return nc


def _get_nc():
    if not _NC_CACHE:
        _NC_CACHE.append(_build())
    return _NC_CACHE[0]


def _make_in_maps(s_o, t_o, ipt, roi_map):
    s_o = np.ascontiguousarray(np.asarray(s_o, dtype=np.float32))
    t_o = np.ascontiguousarray(np.asarray(t_o, dtype=np.float32))
    ipt = np.asarray(ipt, dtype=np.float32)
    roi_map = np.ascontiguousarray(np.asarray(roi_map, dtype=np.float32))
    in_maps = []
    for i in range(N_CORES):
        lo, hi = i * NB, (i + 1) * NB
        in_maps.append(
            {
                "s_o": s_o[lo:hi],
                "t_o": t_o[lo:hi],
                "ipt": np.ascontiguousarray(ipt[lo:hi]).reshape(NR, W, C),
                "roi_map": roi_map,
            }
        )
    return in_maps


def _execute(in_maps, **kwargs):
    nc = _get_nc()
    return run_bass_kernel_spmd(nc, in_maps, core_ids=list(range(N_CORES)), **kwargs)


def kernel(s_o, t_o, ipt, roi_map):
    in_maps = _make_in_maps(s_o, t_o, ipt, roi_map)
    res = _execute(in_maps)
    return np.concatenate(
        [res.results[i]["out"].reshape(NB, H, W, C) for i in range(N_CORES)], axis=0
    )
